# revision 30
# baseline (speedup 1.0000x reference)
"""Trainium2 Bass kernel for nn_AttentionHead (B=4, S=4096, D=512).

reference:
    K = x @ Wk.T; Q = x @ Wq.T; V = x @ Wv.T            # [B,S,D]
    scores[b,s,t] = <K[b,s], Q[b,t]> / sqrt(D)
    scores[b,:,t] = -1e12 where mask[b,t]==0
    out = softmax(scores, axis=t) @ V                    # [B,S,D]

Sharding: 8 cores = 4 batches x 2 sequence halves (rows s of the score
matrix). No collectives; each core computes Q^T/V for the full sequence of
its batch and K^T for its s-half only.

Device dataflow (per core), all matmuls in float32r (full PE rate, ~1e-4
rounding):
    phase 1: Q^T[d,t] = WqT.T-tiles @ x^T        (lhsT=WqT tile, rhs=x^T)
             K^T[d,s] likewise from x^T s-half columns
             V[t,d]   (lhsT = x^T tile, rhs=WvT), scaled by mask[t], with
             mask[t] appended as a 513th feature column (softmax denominator)
    phase 2: per s-chunk of 512: for each t-tile of 128:
             S^T[t,s]  = sum_d Q^T-tile.T @ K^T          (PSUM)
             P^T       = exp(S^T / sqrt(D))              (ACT, -> f32r SBUF)
             out^T[d,s]+= V-tile.T @ P^T  (4 d-tiles + mask col = denominator)
             normalize: out^T *= broadcast(1/den), DMA out.

The masked softmax is computed as exp(scores)*mask / sum(exp(scores)*mask),
which equals softmax with -1e12 masking exactly (masked lanes contribute
exp(-1e12)=0) as long as each batch has at least one unmasked key (holds for
random 0/1 masks over 4096 positions).

Host passes x^T / W^T layouts (pure permutations; all FLOPs stay on device).
"""

import numpy as np

import concourse.bacc as bacc
import concourse.mybir as mybir
from concourse.bass_utils import run_bass_kernel_spmd
from concourse.tile import TileContext

B, S, D = 4, 4096, 512
SH = S // 2          # per-core s rows (half sequence)
P = 128              # partition tile
CH = 512             # free-dim chunk
KD = D // P          # 4 contraction tiles over d
NT = S // P          # 32 t-tiles
SCALE = 1.0 / float(np.sqrt(D))

F32 = mybir.dt.float32
F32R = mybir.dt.float32r
COPY = mybir.ActivationFunctionType.Copy
EXP = mybir.ActivationFunctionType.Exp

VW = D               # V tile width (mask folded into EXP bias instead)

_CACHE = {}


RG = [[0, 1], [2, 3], [4, 5], [6, 7]]   # core pairs sharing one batch


def _build():
    nc = bacc.Bacc(num_devices=8)
    xsT = nc.declare_dram_parameter("xsT", [D, SH], F32R, isOutput=False)
    wqT = nc.declare_dram_parameter("wqT", [D, D], F32R, isOutput=False)
    wkT = nc.declare_dram_parameter("wkT", [D, D], F32R, isOutput=False)
    wvT = nc.declare_dram_parameter("wvT", [D, D], F32R, isOutput=False)
    maskT = nc.declare_dram_parameter("maskT", [P, NT], F32, isOutput=False)
    outT = nc.declare_dram_parameter("outT", [D, SH], F32, isOutput=True)

    # collective bounce buffers: each core computes Q^T/V for its own
    # s-half; the batch pair all-gathers to get the full sequence.
    qh = nc.dram_tensor("qh", [KD * P, SH], F32R)
    qg = nc.dram_tensor("qg", [2 * KD * P, SH], F32R)
    vh = nc.dram_tensor("vh", [SH, VW], F32R)
    vg = nc.dram_tensor("vg", [2 * SH, VW], F32R)

    with TileContext(nc) as tc:
        with tc.tile_pool(name="pers", bufs=1) as pers:
            qT = pers.tile([P, KD * S], F32R)        # d-tile j at [:, j*S:]
            kT = pers.tile([P, KD * SH], F32R)
            vA = pers.tile([P, NT * VW], F32R)       # t-tile i at [:, i*VW:]
            mk = pers.tile([P, NT], F32)
            ones = pers.tile([1, P], F32R)
            ones32 = pers.tile([1, P], F32)
            onec = pers.tile([P, 1], F32R)
            onec32 = pers.tile([P, 1], F32)
            nc.sync.dma_start(out=mk, in_=maskT[:, :])
            nc.vector.memset(ones32, 1.0)
            nc.vector.tensor_copy(out=ones, in_=ones32)
            nc.vector.memset(onec32, 1.0)
            nc.vector.tensor_copy(out=onec, in_=onec32)
            # mbias[p, i] = (mask-1)*1e9: 0 where kept, -1e9 where masked;
            # exp(score*scale + mbias) underflows to exactly 0 on masked keys
            mbias = pers.tile([P, NT], F32)
            nc.vector.tensor_scalar(mbias, mk, -1.0, 1.0e9,
                                    mybir.AluOpType.add,
                                    mybir.AluOpType.mult)

            # ---------------- phase 1: projections ----------------
            with tc.tile_pool(name="stage", bufs=1) as stage, \
                 tc.tile_pool(name="ppsum", bufs=3, space="PSUM") as ppsum:
                wq = stage.tile([P, KD * D], F32R, tag="wq")
                wk = stage.tile([P, KD * D], F32R, tag="wk")
                wv = stage.tile([P, KD * D], F32R, tag="wv")
                for j in range(KD):
                    nc.sync.dma_start(out=wq[:, j * D:(j + 1) * D],
                                      in_=wqT[j * P:(j + 1) * P, :])
                    nc.gpsimd.dma_start(out=wk[:, j * D:(j + 1) * D],
                                        in_=wkT[j * P:(j + 1) * P, :])
                    nc.gpsimd.dma_start(out=wv[:, j * D:(j + 1) * D],
                                        in_=wvT[j * P:(j + 1) * P, :])

                # pass A: Q^T for own s-half -> DRAM qh, all-gather early
                for c in range(SH // CH):
                    xr = stage.tile([P, KD * CH], F32R, tag="xr", bufs=2,
                                    name="xrq")
                    for j in range(KD):
                        nc.sync.dma_start(
                            out=xr[:, j * CH:(j + 1) * CH],
                            in_=xsT[j * P:(j + 1) * P, c * CH:(c + 1) * CH])
                    for jo in range(KD):
                        pq = ppsum.tile([P, CH], F32, tag="pq")
                        for kd in range(KD):
                            nc.tensor.matmul(
                                pq,
                                wq[:, kd * D + jo * P: kd * D + (jo + 1) * P],
                                xr[:, kd * CH:(kd + 1) * CH],
                                start=(kd == 0), stop=(kd == KD - 1))
                        qstg = stage.tile([P, CH], F32R, tag="stg", bufs=2, name="qstg")
                        nc.scalar.activation(out=qstg, in_=pq, func=COPY)
                        nc.sync.dma_start(
                            out=qh[jo * P:(jo + 1) * P, c * CH:(c + 1) * CH],
                            in_=qstg)
                nc.gpsimd.collective_compute(
                    "AllGather", mybir.AluOpType.bypass, replica_groups=RG,
                    ins=[qh[:, :]], outs=[qg[:, :]])

                # pass B: V (-> DRAM vh, all-gather) then K^T (-> SBUF kT)
                for c in range(SH // CH):
                    xr = stage.tile([P, KD * CH], F32R, tag="xr", bufs=2,
                                    name="xrvk")
                    for j in range(KD):
                        nc.sync.dma_start(
                            out=xr[:, j * CH:(j + 1) * CH],
                            in_=xsT[j * P:(j + 1) * P, c * CH:(c + 1) * CH])
                    for tt in range(CH // P):
                        ti = c * (CH // P) + tt
                        pv = ppsum.tile([P, D], F32, tag="pv")
                        for kd in range(KD):
                            nc.tensor.matmul(
                                pv,
                                xr[:, kd * CH + tt * P: kd * CH + (tt + 1) * P],
                                wv[:, kd * D:(kd + 1) * D],
                                start=(kd == 0), stop=(kd == KD - 1))
                        vstg = stage.tile([P, VW], F32R, tag="stg", bufs=2, name="vstg")
                        nc.scalar.activation(out=vstg, in_=pv, func=COPY)
                        nc.sync.dma_start(
                            out=vh[ti * P:(ti + 1) * P, :], in_=vstg)
                    for jo in range(KD):
                        pk = ppsum.tile([P, CH], F32, tag="pq", name="pk")
                        for kd in range(KD):
                            nc.tensor.matmul(
                                pk,
                                wk[:, kd * D + jo * P: kd * D + (jo + 1) * P],
                                xr[:, kd * CH:(kd + 1) * CH],
                                start=(kd == 0), stop=(kd == KD - 1))
                        nc.scalar.activation(
                            out=kT[:, jo * SH + c * CH: jo * SH + (c + 1) * CH],
                            in_=pk, func=COPY)
                nc.gpsimd.collective_compute(
                    "AllGather", mybir.AluOpType.bypass, replica_groups=RG,
                    ins=[vh[:, :]], outs=[vg[:, :]])

                # gather results back to SBUF (r = rank within the pair =
                # which s-half of the sequence)
                for r in range(2):
                    for jo in range(KD):
                        nc.sync.dma_start(
                            out=qT[:, jo * S + r * SH: jo * S + (r + 1) * SH],
                            in_=qg[(r * KD + jo) * P:(r * KD + jo + 1) * P, :])
                for r in range(2):
                    for i in range(SH // P):
                        gi = r * (SH // P) + i
                        eng = nc.sync if i % 2 == 0 else nc.gpsimd
                        eng.dma_start(
                            out=vA[:, gi * VW:(gi + 1) * VW],
                            in_=vg[gi * P:(gi + 1) * P, :])

            # ---------------- phase 2: attention ----------------
            with tc.tile_pool(name="att", bufs=1) as att, \
                 tc.tile_pool(name="apsum", bufs=1, space="PSUM") as apsum:

                for sc in range(SH // CH):
                    opsum = [apsum.tile([P, CH], F32, tag=f"o{d}",
                                        name=f"opsum{d}")
                             for d in range(KD)]
                    # mask weights: P^T sum accumulated on DVE (not PE)
                    den128 = att.tile([P, CH], F32R, tag="den128")

                    def s_group(ti, sc=sc):
                        ss = apsum.tile([P, CH], F32, tag="s", bufs=3)
                        for kd in range(KD):
                            nc.tensor.matmul(
                                ss,
                                qT[:, kd * S + ti * P: kd * S + (ti + 1) * P],
                                kT[:, kd * SH + sc * CH: kd * SH + (sc + 1) * CH],
                                start=(kd == 0), stop=(kd == KD - 1))
                        return ss

                    ss_cur = s_group(0)
                    for ti in range(NT):
                        ss_next = s_group(ti + 1) if ti + 1 < NT else None
                        pt = att.tile([P, CH], F32R, tag="pt", bufs=2)
                        # masked softmax numerator: exp(score*scale + mbias)
                        nc.scalar.activation(out=pt, in_=ss_cur, func=EXP,
                                             scale=SCALE,
                                             bias=mbias[:, ti:ti + 1])
                        for d in range(KD):
                            nc.tensor.matmul(
                                opsum[d],
                                vA[:, ti * VW + d * P: ti * VW + (d + 1) * P],
                                pt, start=(ti == 0), stop=(ti == NT - 1))
                        if ti == 0:
                            nc.vector.tensor_copy(out=den128, in_=pt)
                        else:
                            nc.vector.tensor_add(den128, den128, pt)
                        ss_cur = ss_next

                    # denominator: den[s] = column sum of den128 (P^T already
                    # masked by the EXP bias)
                    dps = apsum.tile([1, CH], F32, tag="bc", name="dps")
                    nc.tensor.matmul(dps, onec, den128, start=True, stop=True)
                    # drain opsum banks via DVE first so PE can reuse them
                    # without waiting on the reciprocal chain
                    osb = []
                    for d in range(KD):
                        ot = att.tile([P, CH], F32, tag=f"osb{d}",
                                      name=f"osb{d}")
                        nc.vector.tensor_copy(out=ot, in_=opsum[d])
                        osb.append(ot)
                    dsb = att.tile([1, CH], F32, tag="dsb")
                    nc.scalar.activation(out=dsb, in_=dps, func=COPY)
                    rec = att.tile([1, CH], F32, tag="rec")
                    nc.vector.reciprocal_approx_fast(out=rec, in_=dsb)
                    recr = att.tile([1, CH], F32R, tag="recr")
                    nc.vector.tensor_copy(out=recr, in_=rec)
                    bps = apsum.tile([P, CH], F32, tag="bc", name="bps")
                    nc.tensor.matmul(bps, ones, recr, start=True, stop=True)
                    bsb = att.tile([P, CH], F32, tag="bsb")
                    nc.scalar.activation(out=bsb, in_=bps, func=COPY)
                    for d in range(KD):
                        fin = att.tile([P, CH], F32, tag="fin", bufs=1)
                        nc.vector.tensor_mul(fin, osb[d], bsb)
                        eng = nc.sync if d % 2 == 0 else nc.gpsimd
                        eng.dma_start(
                            out=outT[d * P:(d + 1) * P, sc * CH:(sc + 1) * CH],
                            in_=fin)

    nc.compile()
    return nc


def kernel(x, mask, Wk, Wq, Wv):
    if "nc" not in _CACHE:
        _CACHE["nc"] = _build()
    nc = _CACHE["nc"]

    x = np.asarray(x, dtype=np.float32)
    mask_f = np.asarray(mask).astype(np.float32)
    wqT = np.ascontiguousarray(np.asarray(Wq, dtype=np.float32).T)
    wkT = np.ascontiguousarray(np.asarray(Wk, dtype=np.float32).T)
    wvT = np.ascontiguousarray(np.asarray(Wv, dtype=np.float32).T)

    in_maps = []
    mks = [np.ascontiguousarray(mask_f[b].reshape(NT, P).T) for b in range(B)]
    for b in range(B):
        for h in range(2):
            in_maps.append({
                "xsT": np.ascontiguousarray(x[b, h * SH:(h + 1) * SH, :].T),
                "wqT": wqT, "wkT": wkT, "wvT": wvT,
                "maskT": mks[b],
            })

    res = run_bass_kernel_spmd(nc, in_maps, core_ids=list(range(8)))

    out = np.empty((B, S, D), dtype=np.float32)
    for b in range(B):
        for h in range(2):
            out[b, h * SH:(h + 1) * SH, :] = res.results[2 * b + h]["outT"].T
    return out


# revision 31
# speedup vs baseline: 1.6374x; 1.6374x over previous
"""Trainium2 Bass kernel for nn_AttentionHead (B=4, S=4096, D=512).

reference:
    K = x @ Wk.T; Q = x @ Wq.T; V = x @ Wv.T            # [B,S,D]
    scores[b,s,t] = <K[b,s], Q[b,t]> / sqrt(D)
    scores[b,:,t] = -1e12 where mask[b,t]==0
    out = softmax(scores, axis=t) @ V                    # [B,S,D]

Sharding: 8 cores = 4 batches x 2 sequence halves (rows s of the score
matrix). No collectives; each core computes Q^T/V for the full sequence of
its batch and K^T for its s-half only.

Device dataflow (per core), all matmuls in float32r (full PE rate, ~1e-4
rounding):
    phase 1: Q^T[d,t] = WqT.T-tiles @ x^T        (lhsT=WqT tile, rhs=x^T)
             K^T[d,s] likewise from x^T s-half columns
             V[t,d]   (lhsT = x^T tile, rhs=WvT), scaled by mask[t], with
             mask[t] appended as a 513th feature column (softmax denominator)
    phase 2: per s-chunk of 512: for each t-tile of 128:
             S^T[t,s]  = sum_d Q^T-tile.T @ K^T          (PSUM)
             P^T       = exp(S^T / sqrt(D))              (ACT, -> f32r SBUF)
             out^T[d,s]+= V-tile.T @ P^T  (4 d-tiles + mask col = denominator)
             normalize: out^T *= broadcast(1/den), DMA out.

The masked softmax is computed as exp(scores)*mask / sum(exp(scores)*mask),
which equals softmax with -1e12 masking exactly (masked lanes contribute
exp(-1e12)=0) as long as each batch has at least one unmasked key (holds for
random 0/1 masks over 4096 positions).

Host passes x^T / W^T layouts (pure permutations; all FLOPs stay on device).
"""

import numpy as np

import concourse.bacc as bacc
import concourse.mybir as mybir
from concourse.bass_utils import run_bass_kernel_spmd
from concourse.tile import TileContext

B, S, D = 4, 4096, 512
SH = S // 2          # per-core s rows (half sequence)
P = 128              # partition tile
CH = 512             # free-dim chunk
KD = D // P          # 4 contraction tiles over d
NT = S // P          # 32 t-tiles
SCALE = 1.0 / float(np.sqrt(D))

F32 = mybir.dt.float32
F32R = mybir.dt.float32r
COPY = mybir.ActivationFunctionType.Copy
EXP = mybir.ActivationFunctionType.Exp

VW = D               # V tile width (mask folded into EXP bias instead)

_CACHE = {}


RG = [[0, 1], [2, 3], [4, 5], [6, 7]]   # core pairs sharing one batch


def _build():
    nc = bacc.Bacc(num_devices=8)
    xT = nc.declare_dram_parameter("xT", [D, S], F32R, isOutput=False)
    xsT = nc.declare_dram_parameter("xsT", [D, SH], F32R, isOutput=False)
    wqT = nc.declare_dram_parameter("wqT", [D, D], F32R, isOutput=False)
    wkT = nc.declare_dram_parameter("wkT", [D, D], F32R, isOutput=False)
    wvT = nc.declare_dram_parameter("wvT", [D, D], F32R, isOutput=False)
    maskT = nc.declare_dram_parameter("maskT", [P, NT], F32, isOutput=False)
    outT = nc.declare_dram_parameter("outT", [D, SH], F32, isOutput=True)

    with TileContext(nc) as tc:
        with tc.tile_pool(name="pers", bufs=1) as pers:
            qT = pers.tile([P, KD * S], F32R)        # d-tile j at [:, j*S:]
            kT = pers.tile([P, KD * SH], F32R)
            vA = pers.tile([P, NT * VW], F32R)       # t-tile i at [:, i*VW:]
            mk = pers.tile([P, NT], F32)
            ones = pers.tile([1, P], F32R)
            ones32 = pers.tile([1, P], F32)
            onec = pers.tile([P, 1], F32R)
            onec32 = pers.tile([P, 1], F32)
            nc.sync.dma_start(out=mk, in_=maskT[:, :])
            nc.vector.memset(ones32, 1.0)
            nc.vector.tensor_copy(out=ones, in_=ones32)
            nc.vector.memset(onec32, 1.0)
            nc.vector.tensor_copy(out=onec, in_=onec32)
            # mbias[p, i] = (mask-1)*1e9: 0 where kept, -1e9 where masked;
            # exp(score*scale + mbias) underflows to exactly 0 on masked keys
            mbias = pers.tile([P, NT], F32)
            nc.vector.tensor_scalar(mbias, mk, -1.0, 1.0e9,
                                    mybir.AluOpType.add,
                                    mybir.AluOpType.mult)

            # ---------------- phase 1: projections ----------------
            with tc.tile_pool(name="stage", bufs=1) as stage, \
                 tc.tile_pool(name="ppsum", bufs=2, space="PSUM") as ppsum:
                wq = stage.tile([P, KD * D], F32R, tag="wq")
                wk = stage.tile([P, KD * D], F32R, tag="wk")
                wv = stage.tile([P, KD * D], F32R, tag="wv")
                for j in range(KD):
                    nc.sync.dma_start(out=wq[:, j * D:(j + 1) * D],
                                      in_=wqT[j * P:(j + 1) * P, :])
                    nc.gpsimd.dma_start(out=wk[:, j * D:(j + 1) * D],
                                        in_=wkT[j * P:(j + 1) * P, :])
                    nc.gpsimd.dma_start(out=wv[:, j * D:(j + 1) * D],
                                        in_=wvT[j * P:(j + 1) * P, :])

                # K^T first (phase 2's first score groups need it earliest)
                for c in range(SH // CH):
                    xr = stage.tile([P, KD * CH], F32R, tag="xr", bufs=2,
                                    name="xrk")
                    for j in range(KD):
                        nc.sync.dma_start(
                            out=xr[:, j * CH:(j + 1) * CH],
                            in_=xsT[j * P:(j + 1) * P, c * CH:(c + 1) * CH])
                    for jo in range(KD):
                        pq = ppsum.tile([P, CH], F32, tag="pq", name="pqk")
                        for kd in range(KD):
                            nc.tensor.matmul(
                                pq,
                                wk[:, kd * D + jo * P: kd * D + (jo + 1) * P],
                                xr[:, kd * CH:(kd + 1) * CH],
                                start=(kd == 0), stop=(kd == KD - 1))
                        nc.scalar.activation(
                            out=kT[:, jo * SH + c * CH: jo * SH + (c + 1) * CH],
                            in_=pq, func=COPY)

                # Q^T and V from full x^T, chunk by chunk
                for c in range(S // CH):
                    xr = stage.tile([P, KD * CH], F32R, tag="xr", bufs=2,
                                    name="xrq")
                    for j in range(KD):
                        nc.sync.dma_start(
                            out=xr[:, j * CH:(j + 1) * CH],
                            in_=xT[j * P:(j + 1) * P, c * CH:(c + 1) * CH])
                    for jo in range(KD):
                        pq = ppsum.tile([P, CH], F32, tag="pq")
                        for kd in range(KD):
                            nc.tensor.matmul(
                                pq,
                                wq[:, kd * D + jo * P: kd * D + (jo + 1) * P],
                                xr[:, kd * CH:(kd + 1) * CH],
                                start=(kd == 0), stop=(kd == KD - 1))
                        nc.scalar.activation(
                            out=qT[:, jo * S + c * CH: jo * S + (c + 1) * CH],
                            in_=pq, func=COPY)
                    for tt in range(CH // P):
                        ti = c * (CH // P) + tt
                        pv = ppsum.tile([P, D], F32, tag="pv")
                        for kd in range(KD):
                            nc.tensor.matmul(
                                pv,
                                xr[:, kd * CH + tt * P: kd * CH + (tt + 1) * P],
                                wv[:, kd * D:(kd + 1) * D],
                                start=(kd == 0), stop=(kd == KD - 1))
                        nc.scalar.activation(
                            out=vA[:, ti * VW: ti * VW + D], in_=pv,
                            func=COPY)

            # ---------------- phase 2: attention ----------------
            with tc.tile_pool(name="att", bufs=1) as att, \
                 tc.tile_pool(name="apsum", bufs=1, space="PSUM") as apsum:

                for sc in range(SH // CH):
                    opsum = [apsum.tile([P, CH], F32, tag=f"o{d}",
                                        name=f"opsum{d}")
                             for d in range(KD)]
                    # mask weights: P^T sum accumulated on DVE (not PE)
                    den128 = att.tile([P, CH], F32R, tag="den128")

                    def s_group(ti, sc=sc):
                        ss = apsum.tile([P, CH], F32, tag="s", bufs=3)
                        for kd in range(KD):
                            nc.tensor.matmul(
                                ss,
                                qT[:, kd * S + ti * P: kd * S + (ti + 1) * P],
                                kT[:, kd * SH + sc * CH: kd * SH + (sc + 1) * CH],
                                start=(kd == 0), stop=(kd == KD - 1))
                        return ss

                    ss_cur = s_group(0)
                    for ti in range(NT):
                        ss_next = s_group(ti + 1) if ti + 1 < NT else None
                        pt = att.tile([P, CH], F32R, tag="pt", bufs=3)
                        # masked softmax numerator: exp(score*scale + mbias)
                        nc.scalar.activation(out=pt, in_=ss_cur, func=EXP,
                                             scale=SCALE,
                                             bias=mbias[:, ti:ti + 1])
                        for d in range(KD):
                            nc.tensor.matmul(
                                opsum[d],
                                vA[:, ti * VW + d * P: ti * VW + (d + 1) * P],
                                pt, start=(ti == 0), stop=(ti == NT - 1))
                        if ti == 0:
                            nc.vector.tensor_copy(out=den128, in_=pt)
                        else:
                            nc.vector.tensor_add(den128, den128, pt)
                        ss_cur = ss_next

                    # denominator: den[s] = column sum of den128 (P^T already
                    # masked by the EXP bias)
                    dps = apsum.tile([1, CH], F32, tag="bc", name="dps")
                    nc.tensor.matmul(dps, onec, den128, start=True, stop=True)
                    # drain opsum banks via DVE first so PE can reuse them
                    # without waiting on the reciprocal chain
                    osb = []
                    for d in range(KD):
                        ot = att.tile([P, CH], F32, tag=f"osb{d}",
                                      name=f"osb{d}")
                        nc.vector.tensor_copy(out=ot, in_=opsum[d])
                        osb.append(ot)
                    dsb = att.tile([1, CH], F32, tag="dsb")
                    nc.scalar.activation(out=dsb, in_=dps, func=COPY)
                    rec = att.tile([1, CH], F32, tag="rec")
                    nc.vector.reciprocal_approx_fast(out=rec, in_=dsb)
                    recr = att.tile([1, CH], F32R, tag="recr")
                    nc.vector.tensor_copy(out=recr, in_=rec)
                    bps = apsum.tile([P, CH], F32, tag="bc", name="bps")
                    nc.tensor.matmul(bps, ones, recr, start=True, stop=True)
                    bsb = att.tile([P, CH], F32, tag="bsb")
                    nc.scalar.activation(out=bsb, in_=bps, func=COPY)
                    for d in range(KD):
                        fin = att.tile([P, CH], F32, tag="fin", bufs=2)
                        nc.vector.tensor_mul(fin, osb[d], bsb)
                        eng = nc.sync if d % 2 == 0 else nc.gpsimd
                        eng.dma_start(
                            out=outT[d * P:(d + 1) * P, sc * CH:(sc + 1) * CH],
                            in_=fin)

    nc.compile()
    return nc


def kernel(x, mask, Wk, Wq, Wv):
    if "nc" not in _CACHE:
        _CACHE["nc"] = _build()
    nc = _CACHE["nc"]

    x = np.asarray(x, dtype=np.float32)
    mask_f = np.asarray(mask).astype(np.float32)
    wqT = np.ascontiguousarray(np.asarray(Wq, dtype=np.float32).T)
    wkT = np.ascontiguousarray(np.asarray(Wk, dtype=np.float32).T)
    wvT = np.ascontiguousarray(np.asarray(Wv, dtype=np.float32).T)

    in_maps = []
    xTs = [np.ascontiguousarray(x[b].T) for b in range(B)]
    mks = [np.ascontiguousarray(mask_f[b].reshape(NT, P).T) for b in range(B)]
    for b in range(B):
        for h in range(2):
            in_maps.append({
                "xT": xTs[b],
                "xsT": np.ascontiguousarray(xTs[b][:, h * SH:(h + 1) * SH]),
                "wqT": wqT, "wkT": wkT, "wvT": wvT,
                "maskT": mks[b],
            })

    res = run_bass_kernel_spmd(nc, in_maps, core_ids=list(range(8)))

    out = np.empty((B, S, D), dtype=np.float32)
    for b in range(B):
        for h in range(2):
            out[b, h * SH:(h + 1) * SH, :] = res.results[2 * b + h]["outT"].T
    return out


# revision 32
# speedup vs baseline: 1.6530x; 1.0095x over previous
"""Trainium2 Bass kernel for nn_AttentionHead (B=4, S=4096, D=512).

reference:
    K = x @ Wk.T; Q = x @ Wq.T; V = x @ Wv.T            # [B,S,D]
    scores[b,s,t] = <K[b,s], Q[b,t]> / sqrt(D)
    scores[b,:,t] = -1e12 where mask[b,t]==0
    out = softmax(scores, axis=t) @ V                    # [B,S,D]

Sharding: 8 cores = 4 batches x 2 sequence halves (rows s of the score
matrix). No collectives; each core computes Q^T/V for the full sequence of
its batch and K^T for its s-half only.

Device dataflow (per core), all matmuls in float32r (full PE rate, ~1e-4
rounding):
    phase 1: Q^T[d,t] = WqT.T-tiles @ x^T        (lhsT=WqT tile, rhs=x^T)
             K^T[d,s] likewise from x^T s-half columns
             V[t,d]   (lhsT = x^T tile, rhs=WvT), scaled by mask[t], with
             mask[t] appended as a 513th feature column (softmax denominator)
    phase 2: per s-chunk of 512: for each t-tile of 128:
             S^T[t,s]  = sum_d Q^T-tile.T @ K^T          (PSUM)
             P^T       = exp(S^T / sqrt(D))              (ACT, -> f32r SBUF)
             out^T[d,s]+= V-tile.T @ P^T  (4 d-tiles + mask col = denominator)
             normalize: out^T *= broadcast(1/den), DMA out.

The masked softmax is computed as exp(scores)*mask / sum(exp(scores)*mask),
which equals softmax with -1e12 masking exactly (masked lanes contribute
exp(-1e12)=0) as long as each batch has at least one unmasked key (holds for
random 0/1 masks over 4096 positions).

Host passes x^T / W^T layouts (pure permutations; all FLOPs stay on device).
"""

import numpy as np

import concourse.bacc as bacc
import concourse.mybir as mybir
from concourse.bass_utils import run_bass_kernel_spmd
from concourse.tile import TileContext

B, S, D = 4, 4096, 512
SH = S // 2          # per-core s rows (half sequence)
P = 128              # partition tile
CH = 512             # free-dim chunk
KD = D // P          # 4 contraction tiles over d
NT = S // P          # 32 t-tiles
SCALE = 1.0 / float(np.sqrt(D))

F32 = mybir.dt.float32
F32R = mybir.dt.float32r
COPY = mybir.ActivationFunctionType.Copy
EXP = mybir.ActivationFunctionType.Exp
BF16 = mybir.dt.bfloat16

VW = D               # V tile width (mask folded into EXP bias instead)

_CACHE = {}


RG = [[0, 1], [2, 3], [4, 5], [6, 7]]   # core pairs sharing one batch


def _build():
    nc = bacc.Bacc(num_devices=8)
    xT = nc.declare_dram_parameter("xT", [D, S], F32R, isOutput=False)
    xsT = nc.declare_dram_parameter("xsT", [D, SH], F32R, isOutput=False)
    wqT = nc.declare_dram_parameter("wqT", [D, D], F32R, isOutput=False)
    wkT = nc.declare_dram_parameter("wkT", [D, D], F32R, isOutput=False)
    wvT = nc.declare_dram_parameter("wvT", [D, D], F32R, isOutput=False)
    maskT = nc.declare_dram_parameter("maskT", [P, NT], F32, isOutput=False)
    outT = nc.declare_dram_parameter("outT", [D, SH], F32, isOutput=True)

    with TileContext(nc) as tc:
        with tc.tile_pool(name="pers", bufs=1) as pers:
            qT = pers.tile([P, KD * S], F32R)        # d-tile j at [:, j*S:]
            kT = pers.tile([P, KD * SH], F32R)
            vA = pers.tile([P, NT * VW], BF16)       # t-tile i at [:, i*VW:]
            mk = pers.tile([P, NT], F32)
            ones = pers.tile([1, P], F32R)
            ones32 = pers.tile([1, P], F32)
            onec = pers.tile([P, 1], F32R)
            onec32 = pers.tile([P, 1], F32)
            nc.sync.dma_start(out=mk, in_=maskT[:, :])
            nc.vector.memset(ones32, 1.0)
            nc.vector.tensor_copy(out=ones, in_=ones32)
            nc.vector.memset(onec32, 1.0)
            nc.vector.tensor_copy(out=onec, in_=onec32)
            # mbias[p, i] = (mask-1)*1e9: 0 where kept, -1e9 where masked;
            # exp(score*scale + mbias) underflows to exactly 0 on masked keys
            mbias = pers.tile([P, NT], F32)
            nc.vector.tensor_scalar(mbias, mk, -1.0, 1.0e9,
                                    mybir.AluOpType.add,
                                    mybir.AluOpType.mult)

            # ---------------- phase 1: projections ----------------
            with tc.tile_pool(name="stage", bufs=1) as stage, \
                 tc.tile_pool(name="ppsum", bufs=2, space="PSUM") as ppsum:
                wq = stage.tile([P, KD * D], F32R, tag="wq")
                wk = stage.tile([P, KD * D], F32R, tag="wk")
                wv = stage.tile([P, KD * D], F32R, tag="wv")
                for j in range(KD):
                    nc.sync.dma_start(out=wq[:, j * D:(j + 1) * D],
                                      in_=wqT[j * P:(j + 1) * P, :])
                    nc.gpsimd.dma_start(out=wk[:, j * D:(j + 1) * D],
                                        in_=wkT[j * P:(j + 1) * P, :])
                    nc.gpsimd.dma_start(out=wv[:, j * D:(j + 1) * D],
                                        in_=wvT[j * P:(j + 1) * P, :])

                # K^T first (phase 2's first score groups need it earliest)
                for c in range(SH // CH):
                    xr = stage.tile([P, KD * CH], F32R, tag="xr", bufs=2,
                                    name="xrk")
                    for j in range(KD):
                        nc.sync.dma_start(
                            out=xr[:, j * CH:(j + 1) * CH],
                            in_=xsT[j * P:(j + 1) * P, c * CH:(c + 1) * CH])
                    for jo in range(KD):
                        pq = ppsum.tile([P, CH], F32, tag="pq", name="pqk")
                        for kd in range(KD):
                            nc.tensor.matmul(
                                pq,
                                wk[:, kd * D + jo * P: kd * D + (jo + 1) * P],
                                xr[:, kd * CH:(kd + 1) * CH],
                                start=(kd == 0), stop=(kd == KD - 1))
                        nc.scalar.activation(
                            out=kT[:, jo * SH + c * CH: jo * SH + (c + 1) * CH],
                            in_=pq, func=COPY)

                # Q^T and V from full x^T, chunk by chunk
                for c in range(S // CH):
                    xr = stage.tile([P, KD * CH], F32R, tag="xr", bufs=2,
                                    name="xrq")
                    for j in range(KD):
                        nc.sync.dma_start(
                            out=xr[:, j * CH:(j + 1) * CH],
                            in_=xT[j * P:(j + 1) * P, c * CH:(c + 1) * CH])
                    for jo in range(KD):
                        pq = ppsum.tile([P, CH], F32, tag="pq")
                        for kd in range(KD):
                            nc.tensor.matmul(
                                pq,
                                wq[:, kd * D + jo * P: kd * D + (jo + 1) * P],
                                xr[:, kd * CH:(kd + 1) * CH],
                                start=(kd == 0), stop=(kd == KD - 1))
                        nc.scalar.activation(
                            out=qT[:, jo * S + c * CH: jo * S + (c + 1) * CH],
                            in_=pq, func=COPY)
                    for tt in range(CH // P):
                        ti = c * (CH // P) + tt
                        pv = ppsum.tile([P, D], F32, tag="pv")
                        for kd in range(KD):
                            nc.tensor.matmul(
                                pv,
                                xr[:, kd * CH + tt * P: kd * CH + (tt + 1) * P],
                                wv[:, kd * D:(kd + 1) * D],
                                start=(kd == 0), stop=(kd == KD - 1))
                        nc.scalar.activation(
                            out=vA[:, ti * VW: ti * VW + D], in_=pv,
                            func=COPY)

            # ---------------- phase 2: attention ----------------
            with tc.tile_pool(name="att", bufs=1) as att, \
                 tc.tile_pool(name="apsum", bufs=1, space="PSUM") as apsum:

                for sc in range(SH // CH):
                    opsum = [apsum.tile([P, CH], F32, tag=f"o{d}",
                                        name=f"opsum{d}")
                             for d in range(KD)]
                    # mask weights: P^T sum accumulated on DVE (not PE)
                    den128 = att.tile([P, CH], F32R, tag="den128")

                    def s_group(ti, sc=sc):
                        ss = apsum.tile([P, CH], F32, tag="s", bufs=3)
                        for kd in range(KD):
                            nc.tensor.matmul(
                                ss,
                                qT[:, kd * S + ti * P: kd * S + (ti + 1) * P],
                                kT[:, kd * SH + sc * CH: kd * SH + (sc + 1) * CH],
                                start=(kd == 0), stop=(kd == KD - 1))
                        return ss

                    ss_cur = s_group(0)
                    for ti in range(NT):
                        ss_next = s_group(ti + 1) if ti + 1 < NT else None
                        pt = att.tile([P, CH], BF16, tag="pt", bufs=3)
                        # masked softmax numerator: exp(score*scale + mbias)
                        nc.scalar.activation(out=pt, in_=ss_cur, func=EXP,
                                             scale=SCALE,
                                             bias=mbias[:, ti:ti + 1])
                        for d in range(KD):
                            nc.tensor.matmul(
                                opsum[d],
                                vA[:, ti * VW + d * P: ti * VW + (d + 1) * P],
                                pt, start=(ti == 0), stop=(ti == NT - 1))
                        if ti == 0:
                            nc.vector.tensor_copy(out=den128, in_=pt)
                        else:
                            nc.vector.tensor_add(den128, den128, pt)
                        ss_cur = ss_next

                    # denominator: den[s] = column sum of den128 (P^T already
                    # masked by the EXP bias)
                    dps = apsum.tile([1, CH], F32, tag="bc", name="dps")
                    nc.tensor.matmul(dps, onec, den128, start=True, stop=True)
                    # drain opsum banks via DVE first so PE can reuse them
                    # without waiting on the reciprocal chain
                    osb = []
                    for d in range(KD):
                        ot = att.tile([P, CH], F32, tag=f"osb{d}",
                                      name=f"osb{d}")
                        nc.vector.tensor_copy(out=ot, in_=opsum[d])
                        osb.append(ot)
                    dsb = att.tile([1, CH], F32, tag="dsb")
                    nc.scalar.activation(out=dsb, in_=dps, func=COPY)
                    rec = att.tile([1, CH], F32, tag="rec")
                    nc.vector.reciprocal_approx_fast(out=rec, in_=dsb)
                    recr = att.tile([1, CH], F32R, tag="recr")
                    nc.vector.tensor_copy(out=recr, in_=rec)
                    bps = apsum.tile([P, CH], F32, tag="bc", name="bps")
                    nc.tensor.matmul(bps, ones, recr, start=True, stop=True)
                    bsb = att.tile([P, CH], F32, tag="bsb")
                    nc.scalar.activation(out=bsb, in_=bps, func=COPY)
                    for d in range(KD):
                        fin = att.tile([P, CH], F32, tag="fin", bufs=2)
                        nc.vector.tensor_mul(fin, osb[d], bsb)
                        eng = nc.sync if d % 2 == 0 else nc.gpsimd
                        eng.dma_start(
                            out=outT[d * P:(d + 1) * P, sc * CH:(sc + 1) * CH],
                            in_=fin)

    nc.compile()
    return nc


def kernel(x, mask, Wk, Wq, Wv):
    if "nc" not in _CACHE:
        _CACHE["nc"] = _build()
    nc = _CACHE["nc"]

    x = np.asarray(x, dtype=np.float32)
    mask_f = np.asarray(mask).astype(np.float32)
    wqT = np.ascontiguousarray(np.asarray(Wq, dtype=np.float32).T)
    wkT = np.ascontiguousarray(np.asarray(Wk, dtype=np.float32).T)
    wvT = np.ascontiguousarray(np.asarray(Wv, dtype=np.float32).T)

    in_maps = []
    xTs = [np.ascontiguousarray(x[b].T) for b in range(B)]
    mks = [np.ascontiguousarray(mask_f[b].reshape(NT, P).T) for b in range(B)]
    for b in range(B):
        for h in range(2):
            in_maps.append({
                "xT": xTs[b],
                "xsT": np.ascontiguousarray(xTs[b][:, h * SH:(h + 1) * SH]),
                "wqT": wqT, "wkT": wkT, "wvT": wvT,
                "maskT": mks[b],
            })

    res = run_bass_kernel_spmd(nc, in_maps, core_ids=list(range(8)))

    out = np.empty((B, S, D), dtype=np.float32)
    for b in range(B):
        for h in range(2):
            out[b, h * SH:(h + 1) * SH, :] = res.results[2 * b + h]["outT"].T
    return out
